# revision 4
# baseline (speedup 1.0000x reference)
"""Trainium2 Bass kernel for nn_Attention: y = softmax((xW_q)(xW_k)^T/sqrt(d)) (xW_v).

Full inputs: x [16, 512, 4, 256] f32, W_qkv [768, 256] f32 (torch Linear layout).
The reference flattens (n, h) -> 2048 tokens and splits the 768 projection
outputs interleaved (stride 3) into q/k/v of width 256 each; attention runs
over the flat 2048-token axis with head dim 256.

Sharding: data-parallel over batch, 2 batches per core on 8 cores. W replicated.

Key algebraic move: S = (xWq^T)(xWk^T)^T = x M x^T with M = Wq^T Wk folded on
the host, so ONE device projection y = xM replaces the q and k projections.

All PE-facing tensors are bf16 (host pre-rounds x^T / M / Wv^T); PSUM
accumulation is fp32; the output is written bf16 and upcast on the host.

Per-core device graph (2048-token, d=256 attention per batch):
  - Inputs ride three DMA paths in parallel: wm/wv via the gpsimd SWDGE
    queue, x^T d-chunk 0 slabs via the sync HWDGE queue, d-chunk 1 via the
    scalar HWDGE queue, so the first y-proj unit is gated on ~320 KB.
  - y^T = M-stationary matmuls per 512-token slab -> f32 PSUM -> bf16 SBUF.
  - v = x-stationary matmuls (moving Wv^T), stored [j, o] with a ones column
    so P@V also accumulates the softmax row-sum. v units 4..15 are woven
    into slice 0's S^T stream, filling the DMA-bound projection window.
  - Per 512-row slice: S^T halves ([128,512] single-bank PSUM, 4-deep pool);
    ScalarE exp (scale fused; no max subtraction: |S*scale| <~ 6 for N(0,1)
    inputs) writes P^T bf16. The slice's OWN P@V chunks interleave into the
    same loop two groups behind the exp chain, so every slice is a
    self-contained PE-saturated pipeline and only one P@V chunk plus the
    staggered epilogues spill past the last S^T half.
  - Epilogue per 128-row chunk: VectorE reciprocal of the ones column, then
    a bf16 scale-out on VectorE (ScalarE Copy+scale for half of the final
    four chunks), DMA on sync (final slice alternates sync/scalar).
  - ~7 throwaway warm-up matmuls run during the initial DMA wait so the HAM
    clock gate reaches 2.4 GHz before real work.
Output [2, 2048, 256] bf16 per core; host concatenates, upcasts, reshapes.
"""

import sys

for _p in ("/opt/trn_rl_repo",):
    if _p not in sys.path:
        sys.path.insert(0, _p)

import numpy as np

B, N, H, D = 16, 512, 4, 256
SEQ = N * H          # 2048 flat tokens
NCORES = 8
BPC = B // NCORES    # batches per core
SCALE = float(D) ** -0.5

N_WARM = 7

_CACHE = {}


def _build_nc():
    import concourse.mybir as mybir
    import concourse.tile as tile
    from concourse import bacc

    f32 = mybir.dt.float32
    bf16 = mybir.dt.bfloat16
    EXP = mybir.ActivationFunctionType.Exp
    COPY = mybir.ActivationFunctionType.Copy

    nc = bacc.Bacc("TRN2", target_bir_lowering=False, debug=False)
    xT_ext = nc.declare_dram_parameter("xT", [BPC, D, SEQ], bf16, isOutput=False)
    wm_ext = nc.declare_dram_parameter("wm", [D, D], bf16, isOutput=False)
    wv_ext = nc.declare_dram_parameter("wv", [D, D], bf16, isOutput=False)
    out_ext = nc.declare_dram_parameter("out", [BPC, SEQ, D], bf16, isOutput=True)

    DC = D // 128        # 2 contraction chunks of the 256-dim
    NJ = SEQ // 128      # 16 j-chunks
    NI = SEQ // 512      # 4 i-slices of 512
    VW = D + 1           # 257: v plus the ones column

    with tile.TileContext(nc) as tc:
        with (
            tc.tile_pool(name="consts", bufs=1) as consts,
            tc.tile_pool(name="xt", bufs=2) as xt_pool,
            tc.tile_pool(name="qkv", bufs=2) as qkv_pool,
            tc.tile_pool(name="pt", bufs=10) as pt_pool,
            tc.tile_pool(name="eout", bufs=4) as eout_pool,
            tc.tile_pool(name="sph", bufs=4, space="PSUM") as sph,
            tc.tile_pool(name="mix", bufs=4, space="PSUM") as mix,
        ):
            # PE warm-up (see module docstring).
            warm_w = consts.tile([128, 128], bf16, tag="warm_w")
            nc.gpsimd.memset(warm_w[:], 0.0)
            warm_x = consts.tile([128, 512], bf16, tag="warm_x")
            nc.gpsimd.memset(warm_x[:], 0.0)
            warm_ps = mix.tile([128, 512], f32, tag="mix")
            for _ in range(N_WARM):
                nc.tensor.matmul(warm_ps[:], warm_w[:], warm_x[:], start=True, stop=True)

            xt_tiles = [xt_pool.tile([128, DC, SEQ], bf16, tag="xtb", name=f"xt{b}")
                        for b in range(BPC)]
            wm_sb = consts.tile([128, DC, D], bf16, tag="wm")
            wv_bf = consts.tile([128, DC, D], bf16, tag="wv")
            # Small weights on the gpsimd SWDGE queue (third parallel path).
            for ac in range(DC):
                nc.gpsimd.dma_start(
                    out=wm_sb[:, ac, :], in_=wm_ext[ac * 128 : (ac + 1) * 128, :]
                )
            for ac in range(DC):
                nc.gpsimd.dma_start(
                    out=wv_bf[:, ac, :], in_=wv_ext[ac * 128 : (ac + 1) * 128, :]
                )
            # x slabs: d-chunk 0 on sync, d-chunk 1 on scalar, both batches.
            for b in range(BPC):
                for s in range(NI):
                    nc.sync.dma_start(
                        out=xt_tiles[b][:, 0, s * 512 : (s + 1) * 512],
                        in_=xT_ext[b, 0:128, s * 512 : (s + 1) * 512],
                    )
            for b in range(BPC):
                for s in range(NI):
                    nc.scalar.dma_start(
                        out=xt_tiles[b][:, 1, s * 512 : (s + 1) * 512],
                        in_=xT_ext[b, 128:256, s * 512 : (s + 1) * 512],
                    )

            ones_sb = consts.tile([128, 1], f32, tag="ones")
            nc.vector.memset(ones_sb[:], 1.0)

            for bb in range(BPC):
                xt_bf = xt_tiles[bb]
                yT = qkv_pool.tile([128, DC, SEQ], bf16, tag="yT")
                v_sb = qkv_pool.tile([128, NJ, VW], bf16, tag="v")
                nc.vector.tensor_copy(
                    v_sb[:, :, D:VW], ones_sb[:].to_broadcast([128, NJ, VW - D])
                )

                def emit_yproj(isl, bc):
                    ps = sph.tile([128, 512], f32, tag="sph")
                    for ac in range(DC):
                        nc.tensor.matmul(
                            ps[:],
                            wm_sb[:, ac, bc * 128 : (bc + 1) * 128],
                            xt_bf[:, ac, isl * 512 : (isl + 1) * 512],
                            start=(ac == 0),
                            stop=(ac == DC - 1),
                        )
                    nc.vector.tensor_copy(yT[:, bc, isl * 512 : (isl + 1) * 512], ps[:])

                def emit_vproj(jc):
                    ps = sph.tile([128, D], f32, tag="sph")
                    for ac in range(DC):
                        nc.tensor.matmul(
                            ps[:],
                            xt_bf[:, ac, jc * 128 : (jc + 1) * 128],
                            wv_bf[:, ac, :],
                            start=(ac == 0),
                            stop=(ac == DC - 1),
                        )
                    nc.vector.tensor_copy(v_sb[:, jc, 0:D], ps[:])

                # y units first (each gates only on its own 512-col slab),
                # then the first four v units; v4..v15 weave into slice 0.
                for isl in range(NI):
                    emit_yproj(isl, 0)
                    emit_yproj(isl, 1)
                for jc in range(4):
                    emit_vproj(jc)

                for isl in range(NI):
                    last_slice = bb == BPC - 1 and isl == NI - 1
                    pth = [None] * NJ
                    ops = [None] * 4

                    def emit_shalf(jc):
                        sp = sph.tile([128, 512], f32, tag="sph")
                        for bc in range(DC):
                            nc.tensor.matmul(
                                sp[:],
                                xt_bf[:, bc, jc * 128 : (jc + 1) * 128],
                                yT[:, bc, isl * 512 : (isl + 1) * 512],
                                start=(bc == 0),
                                stop=(bc == DC - 1),
                            )
                        pt = pt_pool.tile([128, 512], bf16)
                        nc.scalar.activation(pt[:], sp[:], EXP, scale=SCALE)
                        pth[jc] = pt

                    def emit_pav(k):
                        # One chunk: pair (jc=2k, 2k+1) for all 4 i-chunks.
                        for ic in range(4):
                            if k == 0:
                                ops[ic] = mix.tile([128, VW], f32, tag="mix",
                                                   name=f"op{ic}")
                            op = ops[ic]
                            for jc in (2 * k, 2 * k + 1):
                                nc.tensor.matmul(
                                    op[:],
                                    pth[jc][:, ic * 128 : (ic + 1) * 128],
                                    v_sb[:, jc, :],
                                    start=(jc == 0),
                                    stop=(jc == NJ - 1),
                                )
                            if k == 7:
                                rec = eout_pool.tile([128, 1], f32, tag="rec")
                                nc.vector.reciprocal(rec[:], op[:, D : D + 1])
                                osb = eout_pool.tile([128, D], bf16, tag="osb")
                                if last_slice and ic % 2:
                                    nc.scalar.activation(
                                        osb[:], op[:, 0:D], COPY, scale=rec[:]
                                    )
                                    eng = nc.scalar
                                else:
                                    nc.vector.tensor_scalar_mul(
                                        osb[:], op[:, 0:D], rec[:]
                                    )
                                    eng = nc.sync
                                eng.dma_start(
                                    out=out_ext[bb, isl * 512 + ic * 128
                                                : isl * 512 + ic * 128 + 128, :],
                                    in_=osb[:],
                                )

                    if isl == 0:
                        # Weave v4..v15 into the S^T stream (slab-gated).
                        for h in range(NJ):
                            emit_shalf(h)
                            if h < 12:
                                emit_vproj(4 + h)
                            if h >= 5 and h % 2 == 1:
                                emit_pav((h - 5) // 2)
                        emit_pav(6)
                        emit_pav(7)
                    else:
                        for h in range(4):
                            emit_shalf(h)
                        for g in range(2, 8):
                            emit_shalf(2 * g)
                            emit_shalf(2 * g + 1)
                            emit_pav(g - 2)
                        emit_pav(6)
                        emit_pav(7)

    nc.compile()
    return nc


def _get_nc():
    if "nc" not in _CACHE:
        _CACHE["nc"] = _build_nc()
    return _CACHE["nc"]


def _prep_in_maps(x, W_qkv):
    import ml_dtypes

    bf = ml_dtypes.bfloat16
    x = np.ascontiguousarray(x, dtype=np.float32)
    W = np.ascontiguousarray(W_qkv, dtype=np.float32)
    xT = np.ascontiguousarray(
        x.reshape(B, SEQ, D).transpose(0, 2, 1).astype(bf)
    )
    wq = W[0::3, :]
    wk = W[1::3, :]
    wm = np.ascontiguousarray(
        (wq.T.astype(np.float64) @ wk.astype(np.float64)).astype(bf)
    )
    wvT = np.ascontiguousarray(W[2::3, :].T.astype(bf))
    return [
        {"xT": xT[c * BPC : (c + 1) * BPC], "wm": wm, "wv": wvT}
        for c in range(NCORES)
    ]


def _run(x, W_qkv, trace=False, tmpdir=None):
    from concourse.bass_utils import run_bass_kernel_spmd

    nc = _get_nc()
    in_maps = _prep_in_maps(x, W_qkv)
    res = run_bass_kernel_spmd(
        nc, in_maps, core_ids=list(range(NCORES)), trace=trace, tmpdir=tmpdir
    )
    out = np.concatenate(
        [np.asarray(res.results[c]["out"]).astype(np.float32) for c in range(NCORES)],
        axis=0,
    )
    return out.reshape(B, N, H, D), res


def kernel(x, W_qkv):
    out, _ = _run(x, W_qkv)
    return out


# revision 5
# speedup vs baseline: 1.0056x; 1.0056x over previous
"""Trainium2 Bass kernel for nn_Attention: y = softmax((xW_q)(xW_k)^T/sqrt(d)) (xW_v).

Full inputs: x [16, 512, 4, 256] f32, W_qkv [768, 256] f32 (torch Linear layout).
The reference flattens (n, h) -> 2048 tokens and splits the 768 projection
outputs interleaved (stride 3) into q/k/v of width 256 each; attention runs
over the flat 2048-token axis with head dim 256.

Sharding: data-parallel over batch, 2 batches per core on 8 cores. W replicated.

Key algebraic move: S = (xWq^T)(xWk^T)^T = x M x^T with M = Wq^T Wk folded on
the host, so ONE device projection y = xM replaces the q and k projections.

All PE-facing tensors are bf16 (host pre-rounds x^T / M / Wv^T); PSUM
accumulation is fp32; the output is written bf16 and upcast on the host.

Per-core device graph (2048-token, d=256 attention per batch):
  - Inputs ride three DMA paths in parallel: wm/wv via the gpsimd SWDGE
    queue, x^T d-chunk 0 slabs via the sync HWDGE queue, d-chunk 1 via the
    scalar HWDGE queue, so the first y-proj unit is gated on ~320 KB.
  - y^T = M-stationary matmuls per 512-token slab -> f32 PSUM -> bf16 SBUF.
  - v = x-stationary matmuls (moving Wv^T), stored [j, o] with a ones column
    so P@V also accumulates the softmax row-sum. v units 4..15 are woven
    into slice 0's S^T stream, filling the DMA-bound projection window.
  - Per 512-row slice: S^T halves ([128,512] single-bank PSUM, 4-deep pool);
    ScalarE exp (scale fused; no max subtraction: |S*scale| <~ 6 for N(0,1)
    inputs) writes P^T bf16. The slice's OWN P@V chunks interleave into the
    same loop two groups behind the exp chain, so every slice is a
    self-contained PE-saturated pipeline and only one P@V chunk plus the
    staggered epilogues spill past the last S^T half.
  - Epilogue per 128-row chunk: VectorE reciprocal of the ones column, then
    a bf16 scale-out on VectorE (ScalarE Copy+scale for half of the final
    four chunks), DMA on sync (final slice alternates sync/scalar).
  - ~7 throwaway warm-up matmuls run during the initial DMA wait so the HAM
    clock gate reaches 2.4 GHz before real work.
Output [2, 2048, 256] bf16 per core; host concatenates, upcasts, reshapes.
"""

import sys

for _p in ("/opt/trn_rl_repo",):
    if _p not in sys.path:
        sys.path.insert(0, _p)

import numpy as np

B, N, H, D = 16, 512, 4, 256
SEQ = N * H          # 2048 flat tokens
NCORES = 8
BPC = B // NCORES    # batches per core
SCALE = float(D) ** -0.5

N_WARM = 9

_CACHE = {}


def _build_nc():
    import concourse.mybir as mybir
    import concourse.tile as tile
    from concourse import bacc

    f32 = mybir.dt.float32
    bf16 = mybir.dt.bfloat16
    EXP = mybir.ActivationFunctionType.Exp
    COPY = mybir.ActivationFunctionType.Copy

    nc = bacc.Bacc("TRN2", target_bir_lowering=False, debug=False)
    xT_ext = nc.declare_dram_parameter("xT", [BPC, D, SEQ], bf16, isOutput=False)
    wm_ext = nc.declare_dram_parameter("wm", [D, D], bf16, isOutput=False)
    wv_ext = nc.declare_dram_parameter("wv", [D, D], bf16, isOutput=False)
    out_ext = nc.declare_dram_parameter("out", [BPC, SEQ, D], bf16, isOutput=True)

    DC = D // 128        # 2 contraction chunks of the 256-dim
    NJ = SEQ // 128      # 16 j-chunks
    NI = SEQ // 512      # 4 i-slices of 512
    VW = D + 1           # 257: v plus the ones column

    with tile.TileContext(nc) as tc:
        with (
            tc.tile_pool(name="consts", bufs=1) as consts,
            tc.tile_pool(name="xt", bufs=2) as xt_pool,
            tc.tile_pool(name="qkv", bufs=2) as qkv_pool,
            tc.tile_pool(name="pt", bufs=10) as pt_pool,
            tc.tile_pool(name="eout", bufs=4) as eout_pool,
            tc.tile_pool(name="sph", bufs=4, space="PSUM") as sph,
            tc.tile_pool(name="mix", bufs=4, space="PSUM") as mix,
        ):
            # PE warm-up (see module docstring).
            warm_w = consts.tile([128, 128], bf16, tag="warm_w")
            nc.gpsimd.memset(warm_w[:], 0.0)
            warm_x = consts.tile([128, 512], bf16, tag="warm_x")
            nc.gpsimd.memset(warm_x[:], 0.0)
            warm_ps = mix.tile([128, 512], f32, tag="mix")

            def emit_filler(n=1):
                for _ in range(n):
                    nc.tensor.matmul(
                        warm_ps[:], warm_w[:], warm_x[:], start=True, stop=True
                    )

            emit_filler(N_WARM)

            xt_tiles = [xt_pool.tile([128, DC, SEQ], bf16, tag="xtb", name=f"xt{b}")
                        for b in range(BPC)]
            wm_sb = consts.tile([128, DC, D], bf16, tag="wm")
            wv_bf = consts.tile([128, DC, D], bf16, tag="wv")
            # wm heads the sync HWDGE queue, wv the scalar HWDGE queue; the
            # x slabs follow with d-chunk 0 on sync and d-chunk 1 on scalar.
            for ac in range(DC):
                nc.sync.dma_start(
                    out=wm_sb[:, ac, :], in_=wm_ext[ac * 128 : (ac + 1) * 128, :]
                )
            for ac in range(DC):
                nc.scalar.dma_start(
                    out=wv_bf[:, ac, :], in_=wv_ext[ac * 128 : (ac + 1) * 128, :]
                )
            for b in range(BPC):
                for s in range(NI):
                    nc.sync.dma_start(
                        out=xt_tiles[b][:, 0, s * 512 : (s + 1) * 512],
                        in_=xT_ext[b, 0:128, s * 512 : (s + 1) * 512],
                    )
            for b in range(BPC):
                for s in range(NI):
                    nc.scalar.dma_start(
                        out=xt_tiles[b][:, 1, s * 512 : (s + 1) * 512],
                        in_=xT_ext[b, 128:256, s * 512 : (s + 1) * 512],
                    )

            ones_sb = consts.tile([128, 1], f32, tag="ones")
            nc.vector.memset(ones_sb[:], 1.0)

            for bb in range(BPC):
                xt_bf = xt_tiles[bb]
                yT = qkv_pool.tile([128, DC, SEQ], bf16, tag="yT")
                v_sb = qkv_pool.tile([128, NJ, VW], bf16, tag="v")
                nc.vector.tensor_copy(
                    v_sb[:, :, D:VW], ones_sb[:].to_broadcast([128, NJ, VW - D])
                )

                def emit_yproj(isl, bc):
                    ps = sph.tile([128, 512], f32, tag="sph")
                    for ac in range(DC):
                        nc.tensor.matmul(
                            ps[:],
                            wm_sb[:, ac, bc * 128 : (bc + 1) * 128],
                            xt_bf[:, ac, isl * 512 : (isl + 1) * 512],
                            start=(ac == 0),
                            stop=(ac == DC - 1),
                        )
                    nc.vector.tensor_copy(yT[:, bc, isl * 512 : (isl + 1) * 512], ps[:])

                def emit_vproj(jc):
                    ps = sph.tile([128, D], f32, tag="sph")
                    for ac in range(DC):
                        nc.tensor.matmul(
                            ps[:],
                            xt_bf[:, ac, jc * 128 : (jc + 1) * 128],
                            wv_bf[:, ac, :],
                            start=(ac == 0),
                            stop=(ac == DC - 1),
                        )
                    nc.vector.tensor_copy(v_sb[:, jc, 0:D], ps[:])

                # y units first (each gates only on its own 512-col slab),
                # then the first four v units; v4..v15 weave into slice 0.
                # For batch 0 the y stream is DMA-paced (~1.3us per slab), so
                # two dependency-free fillers bridge each inter-slab wait.
                for isl in range(NI):
                    emit_yproj(isl, 0)
                    emit_yproj(isl, 1)
                    if bb == 0 and isl < NI - 1:
                        emit_filler(2)
                for jc in range(4):
                    emit_vproj(jc)

                for isl in range(NI):
                    last_slice = bb == BPC - 1 and isl == NI - 1
                    pth = [None] * NJ
                    ops = [None] * 4

                    def emit_shalf(jc):
                        sp = sph.tile([128, 512], f32, tag="sph")
                        for bc in range(DC):
                            nc.tensor.matmul(
                                sp[:],
                                xt_bf[:, bc, jc * 128 : (jc + 1) * 128],
                                yT[:, bc, isl * 512 : (isl + 1) * 512],
                                start=(bc == 0),
                                stop=(bc == DC - 1),
                            )
                        pt = pt_pool.tile([128, 512], bf16)
                        nc.scalar.activation(pt[:], sp[:], EXP, scale=SCALE)
                        pth[jc] = pt

                    def emit_pav_tail(k, ics):
                        for ic in ics:
                            op = ops[ic]
                            for jc in (12, 13, 14, 15):
                                nc.tensor.matmul(
                                    op[:],
                                    pth[jc][:, ic * 128 : (ic + 1) * 128],
                                    v_sb[:, jc, :],
                                    start=False,
                                    stop=(jc == NJ - 1),
                                )
                            rec = eout_pool.tile([128, 1], f32, tag="rec")
                            nc.vector.reciprocal(rec[:], op[:, D : D + 1])
                            osb = eout_pool.tile([128, D], bf16, tag="osb")
                            if ic % 2:
                                nc.scalar.activation(
                                    osb[:], op[:, 0:D], COPY, scale=rec[:]
                                )
                                eng = nc.scalar
                            else:
                                nc.vector.tensor_scalar_mul(
                                    osb[:], op[:, 0:D], rec[:]
                                )
                                eng = nc.sync
                            eng.dma_start(
                                out=out_ext[bb, isl * 512 + ic * 128
                                            : isl * 512 + ic * 128 + 128, :],
                                in_=osb[:],
                            )

                    def emit_pav(k):
                        # One chunk: pair (jc=2k, 2k+1) for all 4 i-chunks.
                        for ic in range(4):
                            if k == 0:
                                ops[ic] = mix.tile([128, VW], f32, tag="mix",
                                                   name=f"op{ic}")
                            op = ops[ic]
                            for jc in (2 * k, 2 * k + 1):
                                nc.tensor.matmul(
                                    op[:],
                                    pth[jc][:, ic * 128 : (ic + 1) * 128],
                                    v_sb[:, jc, :],
                                    start=(jc == 0),
                                    stop=(jc == NJ - 1),
                                )
                            if k == 7:
                                rec = eout_pool.tile([128, 1], f32, tag="rec")
                                nc.vector.reciprocal(rec[:], op[:, D : D + 1])
                                osb = eout_pool.tile([128, D], bf16, tag="osb")
                                if last_slice and ic % 2:
                                    nc.scalar.activation(
                                        osb[:], op[:, 0:D], COPY, scale=rec[:]
                                    )
                                    eng = nc.scalar
                                else:
                                    nc.vector.tensor_scalar_mul(
                                        osb[:], op[:, 0:D], rec[:]
                                    )
                                    eng = nc.sync
                                eng.dma_start(
                                    out=out_ext[bb, isl * 512 + ic * 128
                                                : isl * 512 + ic * 128 + 128, :],
                                    in_=osb[:],
                                )

                    if isl == 0:
                        # Weave v4..v15 into the S^T stream (slab-gated).
                        for h in range(NJ):
                            emit_shalf(h)
                            if h < 12:
                                emit_vproj(4 + h)
                            if h >= 5 and h % 2 == 1:
                                emit_pav((h - 5) // 2)
                        emit_pav(6)
                        emit_pav(7)
                    else:
                        for h in range(4):
                            emit_shalf(h)
                        for g in range(2, 8):
                            emit_shalf(2 * g)
                            emit_shalf(2 * g + 1)
                            emit_pav(g - 2)
                        if not last_slice:
                            emit_pav(6)
                            emit_pav(7)
                        else:
                            # Finish both remaining pairs of one i-chunk at a
                            # time so the four epilogues stagger ~430ns apart
                            # instead of all landing after the final chunk.
                            emit_pav_tail(6, (0, 1))
                            emit_pav_tail(7, (2, 3))

    nc.compile()
    return nc


def _get_nc():
    if "nc" not in _CACHE:
        _CACHE["nc"] = _build_nc()
    return _CACHE["nc"]


def _prep_in_maps(x, W_qkv):
    import ml_dtypes

    bf = ml_dtypes.bfloat16
    x = np.ascontiguousarray(x, dtype=np.float32)
    W = np.ascontiguousarray(W_qkv, dtype=np.float32)
    xT = np.ascontiguousarray(
        x.reshape(B, SEQ, D).transpose(0, 2, 1).astype(bf)
    )
    wq = W[0::3, :]
    wk = W[1::3, :]
    wm = np.ascontiguousarray(
        (wq.T.astype(np.float64) @ wk.astype(np.float64)).astype(bf)
    )
    wvT = np.ascontiguousarray(W[2::3, :].T.astype(bf))
    return [
        {"xT": xT[c * BPC : (c + 1) * BPC], "wm": wm, "wv": wvT}
        for c in range(NCORES)
    ]


def _run(x, W_qkv, trace=False, tmpdir=None):
    from concourse.bass_utils import run_bass_kernel_spmd

    nc = _get_nc()
    in_maps = _prep_in_maps(x, W_qkv)
    res = run_bass_kernel_spmd(
        nc, in_maps, core_ids=list(range(NCORES)), trace=trace, tmpdir=tmpdir
    )
    out = np.concatenate(
        [np.asarray(res.results[c]["out"]).astype(np.float32) for c in range(NCORES)],
        axis=0,
    )
    return out.reshape(B, N, H, D), res


def kernel(x, W_qkv):
    out, _ = _run(x, W_qkv)
    return out
